# revision 6
# baseline (speedup 1.0000x reference)
"""Trainium2 Bass kernel v2 for nn_LSTMModel: single fused NEFF.

Layer-pipelined across core pairs: even core 2g runs LSTM layer 1 for batch
group g (16 rows), odd core 2g+1 runs layer 2 for the same rows, lagging two
32-step blocks. h1 blocks move between pair cores via a per-block 2-core
AllGather (DRAM), with a DRAM->DRAM bounce into a unified "reg" region from
which every core fetches its per-block GEMM input by an indirect DMA whose
row offsets are per-core *data* (even cores point at their own staged eT
blocks, odd cores at the collective landing slot) — keeping the SPMD
program identical across cores.

Per step the recurrent matmul is weight-stationary over U tiles in kc-major
order (all 16 gate tiles for hidden chunk 0, then chunk 1, ...), and the
gate elementwise is split into two hidden-chunk halves so the next step's
first kc passes overlap the second half's computation.

The head (global max pool -> dense+relu -> softmax over 50000) is fused in:
AllGather of the pooled h2 over all 8 cores, redundant dense, vocab-sharded
exp+partial sums, AllReduce of the sums, local scaling. Output: the core's
[64, 6250] probability shard.

All matmuls bf16 with fp32 PSUM. Biases are all zero (asserted).
"""

import numpy as np
import ml_dtypes

import concourse.bass as bass
import concourse.bacc as bacc
import concourse.mybir as mybir
import concourse.tile as tile
from concourse.masks import make_identity

bf16 = mybir.dt.bfloat16
f32 = mybir.dt.float32
i32 = mybir.dt.int32
AF = mybir.ActivationFunctionType
ALU = mybir.AluOpType
bf = ml_dtypes.bfloat16

B, T, V, D, M = 64, 512, 50000, 128, 512
NC = 8
BGRP = 4                 # batch groups (core pairs)
BL = B // BGRP           # 16 batch rows per pair
VS = V // NC             # 6250 vocab cols per core
SB = 32                  # steps per block
KC = M // 128            # 4 hidden chunks
MC = 4 * M // 128        # 16 gate chunks
NBLK = T // SB           # 16 blocks
NSUP = NBLK + 2          # 18 supersteps (odd cores lag by 2)
NTOK = BL * T            # 8192 tokens per group
NGATH = NTOK // 128      # 64 gather tiles
GPB = NGATH // NBLK      # 4 gather tiles per block
BW = KC * SB * BL        # 2048: cols of one block buffer [128, kc, s, b]
ZROW = NBLK * 128        # reg row base of the zero block
CCROW = (NBLK + 1) * 128  # reg row base of cc landing slots (2 x 128 rows)
NREG = CCROW + 2 * 128


def _new_nc():
    return bacc.Bacc("TRN2", target_bir_lowering=False, debug=False, num_devices=NC)


def build_fused(reps=1, no_cc=False):
    NCH = (VS + 511) // 512
    nc = _new_nc()
    ids_d = nc.dram_tensor("ids", [128, NGATH], i32, kind="ExternalInput")
    emb_d = nc.dram_tensor("emb", [V, D], f32, kind="ExternalInput")
    goff_d = nc.dram_tensor("goff", [128, NSUP], i32, kind="ExternalInput")
    u_d = nc.dram_tensor("ut", [128, KC * MC * 128], bf16, kind="ExternalInput")
    w_d = nc.dram_tensor("wt", [128, KC * MC * 128], bf16, kind="ExternalInput")
    wd_d = nc.dram_tensor("wdt", [128, KC * KC * 128], bf16, kind="ExternalInput")
    wo_d = nc.dram_tensor("wot", [128, KC * VS], bf16, kind="ExternalInput")
    pr_d = nc.dram_tensor("probs", [B, VS], f32, kind="ExternalOutput")

    reg = nc.dram_tensor("reg", [NREG, BW], bf16, kind="Internal")
    cin_d = nc.dram_tensor("cc_in", [2 * 128, BW], bf16, kind="Internal")
    cout_d = [nc.dram_tensor(f"cc_out{i}", [2 * 128, BW], bf16, kind="Internal")
              for i in range(2)]
    pin_d = nc.dram_tensor("p_in", [128, KC * BL], f32, kind="Internal")
    pout_d = nc.dram_tensor("p_all", [NC * 128, KC * BL], f32, kind="Internal")
    sin_d = nc.dram_tensor("s_in", [B, 1], f32, kind="Internal")
    sout_d = nc.dram_tensor("s_all", [B, 1], f32, kind="Internal")

    pair_groups = [[2 * g, 2 * g + 1] for g in range(BGRP)]
    all_groups = [list(range(NC))]

    with tile.TileContext(nc) as tc:
        with tc.tile_pool(name="wts", bufs=1) as wpool, \
             tc.tile_pool(name="sb", bufs=3) as pool, \
             tc.tile_pool(name="st", bufs=2) as spool, \
             tc.tile_pool(name="ps", bufs=2, space="PSUM") as psp, \
             tc.tile_pool(name="pst", bufs=1, space="PSUM") as pst:
          for _rep in range(reps):
            u = wpool.tile([128, KC * MC * 128], bf16, tag="u")
            w = wpool.tile([128, KC * MC * 128], bf16, tag="w")
            wd = wpool.tile([128, KC * KC * 128], bf16, tag="wd")
            hist = [wpool.tile([128, BW], bf16, tag=f"hist{i}", name=f"hist{i}")
                    for i in range(2)]
            xw = [wpool.tile([128, MC * SB * BL], bf16, tag=f"xw{i}",
                             name=f"xw{i}") for i in range(2)]
            gemin = [wpool.tile([128, BW], bf16, tag=f"gemin{i}",
                                name=f"gemin{i}") for i in range(2)]
            cst = wpool.tile([128, KC * BL], f32, tag="cst")
            maxp = wpool.tile([128, KC * BL], f32, tag="maxp")
            zero = wpool.tile([128, BW], bf16, tag="zero")
            ident = wpool.tile([128, 128], f32, tag="ident")
            ids_t = wpool.tile([128, NGATH], i32, tag="ids")
            goff_t = wpool.tile([128, NSUP], i32, tag="goff")

            nc.sync.dma_start(u[:], u_d[:])
            nc.sync.dma_start(w[:], w_d[:])
            nc.sync.dma_start(wd[:], wd_d[:])
            nc.sync.dma_start(ids_t[:], ids_d[:])
            nc.sync.dma_start(goff_t[:], goff_d[:])
            nc.vector.memset(cst[:], 0.0)
            nc.vector.memset(maxp[:], 0.0)
            nc.vector.memset(zero[:], 0.0)
            nc.vector.memset(hist[1][:], 0.0)   # h_{-1} = 0 slot
            make_identity(nc, ident[:])
            ident_b = wpool.tile([128, 128], bf16, tag='identb')
            nc.scalar.copy(ident_b[:], ident[:])

            # reg zero block + first eT block + its pad area
            nc.sync.dma_start(reg[ZROW:ZROW + 128, :], zero[:])

            def stage_block(j):
                """Gather+transpose eT block j into reg rows [j*128, +128)."""
                for t in range(GPB):
                    et = pool.tile([128, 128], f32, tag="gath")
                    g = j * GPB + t
                    nc.gpsimd.indirect_dma_start(
                        out=et[:], out_offset=None, in_=emb_d[:],
                        in_offset=bass.IndirectOffsetOnAxis(
                            ap=ids_t[:, g:g + 1], axis=0))
                    tp = pst.tile([128, 128], f32, tag="tp")
                    nc.tensor.transpose(out=tp[:], in_=et[:], identity=ident[:])
                    sg = pool.tile([128, 128], bf16, tag="sg")
                    nc.scalar.copy(sg[:], tp[:])
                    nc.sync.dma_start(
                        reg[j * 128:(j + 1) * 128, t * 128:(t + 1) * 128], sg[:])
                # zero-fill kc chunks 1..3 of the block (never real data there)
                nc.sync.dma_start(
                    reg[j * 128:(j + 1) * 128, SB * BL:BW],
                    zero[:, 0:BW - SB * BL])

            stage_block(0)
            stage_block(1)

            hist_v = [h[:].rearrange("p (j s b) -> p j s b", j=KC, s=SB)
                      for h in hist]
            xw_v = [x[:].rearrange("p (j g s b) -> p j g s b", j=KC, g=4, s=SB)
                    for x in xw]
            gem_v = [x[:].rearrange("p (j s b) -> p j s b", j=KC, s=SB)
                     for x in gemin]
            cst_v = cst[:].rearrange("p (j b) -> p j b", j=KC)
            maxp_v = maxp[:].rearrange("p (j b) -> p j b", j=KC)

            def gather(s):
                nc.gpsimd.indirect_dma_start(
                    out=gemin[s % 2][:], out_offset=None, in_=reg[:, :],
                    in_offset=bass.IndirectOffsetOnAxis(
                        ap=goff_t[:, s:s + 1], axis=0))

            gather(0)

            def gemm_units(s):
                """Emit-list of closures: 4 kc-MMs + 1 copy per mc chunk."""
                gi = s % 2
                units = []
                state = {}
                for mc in range(MC):
                    def mk(mc):
                        def mm(kc):
                            def f():
                                if kc == 0:
                                    state[mc] = psp.tile(
                                        [128, SB * BL], f32, tag="gemm",
                                        name=f"gp_{mc}")
                                nc.tensor.matmul(
                                    state[mc][:],
                                    w[:, (kc * MC + mc) * 128:
                                      (kc * MC + mc + 1) * 128],
                                    gem_v[gi][:, kc, :, :],
                                    start=(kc == 0), stop=(kc == KC - 1))
                            return f
                        def cp():
                            nc.scalar.copy(
                                xw_v[gi][:, mc % KC, mc // KC, :, :],
                                state[mc][:].rearrange("p (s b) -> p s b", s=SB))
                        return [mm(kc) for kc in range(KC)] + [cp]
                    units.extend(mk(mc))
                return units

            # pass-4 mc order: A-half hidden chunks (j in 0,1) first
            P4 = [g * KC + j for j in (0, 1) for g in range(4)] + \
                 [g * KC + j for j in (2, 3) for g in range(4)]

            def lstm_step(s, st, fill):
                gi = s % 2
                if st == 0:
                    hp = hist_v[1 - gi]
                    hrow = SB - 1
                else:
                    hp = hist_v[gi]
                    hrow = st - 1
                zp = psp.tile([128, MC * BL], f32, tag="zp")
                zp_v = zp[:].rearrange("p (j g b) -> p j g b", j=KC, g=4)
                # initialize PSUM with xw via identity matmuls (no deps on h)
                for (j0, j1) in ((0, 2), (2, 4)):
                    nc.tensor.matmul(
                        zp[:, j0 * 4 * BL:j1 * 4 * BL], ident_b[:],
                        xw_v[gi][:, j0:j1, :, st, :], start=True, stop=False)
                for kc in range(KC):
                    order = range(MC) if kc < KC - 1 else P4
                    for mc in order:
                        g, j = mc // KC, mc % KC
                        nc.tensor.matmul(
                            zp_v[:, j, g, :],
                            u[:, (kc * MC + mc) * 128:(kc * MC + mc + 1) * 128],
                            hp[:, kc, hrow, :],
                            start=False, stop=(kc == KC - 1))
                for (j0, j1) in ((0, 2), (2, 4)):
                    jn = j1 - j0
                    sig = spool.tile([128, 3 * jn * BL], f32, tag=f"sig{j0}")
                    sig_v = sig[:].rearrange("p (j g b) -> p j g b", j=jn, g=3)
                    nc.scalar.activation(sig_v[:, :, :, :],
                                         zp_v[:, j0:j1, 0:3, :], AF.Sigmoid)
                    ig = spool.tile([128, jn * BL], f32, tag=f"ig{j0}")
                    ig_v = ig[:].rearrange("p (j b) -> p j b", j=jn)
                    nc.vector.scalar_tensor_tensor(
                        out=ig_v[:, :, :], in0=zp_v[:, j0:j1, 3, :], scalar=0.0,
                        in1=sig_v[:, :, 0, :], op0=ALU.max, op1=ALU.mult)
                    fc = spool.tile([128, jn * BL], f32, tag=f"fc{j0}")
                    fc_v = fc[:].rearrange("p (j b) -> p j b", j=jn)
                    nc.gpsimd.tensor_tensor(
                        out=fc_v[:, :, :], in0=sig_v[:, :, 1, :],
                        in1=cst_v[:, j0:j1, :], op=ALU.mult)
                    nc.vector.tensor_tensor(
                        out=cst_v[:, j0:j1, :], in0=fc_v[:, :, :],
                        in1=ig_v[:, :, :], op=ALU.add)
                    nc.vector.scalar_tensor_tensor(
                        out=hist_v[gi][:, j0:j1, st, :],
                        in0=cst_v[:, j0:j1, :], scalar=0.0,
                        in1=sig_v[:, :, 2, :], op0=ALU.max, op1=ALU.mult)
                for f in fill:
                    f()

            FILL_ST = 8  # first step that takes next-superstep GEMM fill work

            units0 = gemm_units(0)
            for f in units0:
                f()

            for s in range(NSUP):
                gi = s % 2
                if s + 1 < NSUP:
                    gather(s + 1)
                    units = gemm_units(s + 1)
                else:
                    units = []
                nfill = len(units)
                per = -(-nfill // (SB - FILL_ST)) if nfill else 0
                for st in range(SB):
                    if st < FILL_ST or not units:
                        fill = []
                    else:
                        k = (st - FILL_ST) * per
                        fill = units[k:k + per]
                    lstm_step(s, st, fill)
                # fold this block's h into the running max (pool over s)
                bmax = spool.tile([128, KC * BL], f32, tag="bmax")
                nc.vector.tensor_reduce(
                    bmax[:].rearrange("p (j b) -> p j b", j=KC),
                    hist[gi][:].rearrange("p (j s b) -> p j b s", j=KC, s=SB),
                    axis=mybir.AxisListType.X, op=ALU.max)
                nc.vector.tensor_tensor(out=maxp[:], in0=maxp[:],
                                        in1=bmax[:], op=ALU.max)
                if s < NBLK:
                    nc.sync.dma_start(cin_d[gi * 128:(gi + 1) * 128, :],
                                      hist[gi][:])
                    if no_cc:
                        nc.sync.dma_start(cout_d[gi][0:128, :],
                                          cin_d[gi * 128:(gi + 1) * 128, :])
                        nc.sync.dma_start(cout_d[gi][128:256, :],
                                          cin_d[gi * 128:(gi + 1) * 128, :])
                    else:
                        nc.gpsimd.collective_compute(
                            "AllGather", ALU.bypass, replica_groups=pair_groups,
                            ins=[cin_d[gi * 128:(gi + 1) * 128, :]],
                            outs=[cout_d[gi][:, :]])
                if s + 2 < NBLK:
                    stage_block(s + 2)
                if s < NBLK:
                    # bounce rank-0 slice of CC_s into the reg landing slot;
                    # consumed by the partner's gather at superstep s+2
                    nc.sync.dma_start(
                        reg[CCROW + gi * 128:CCROW + (gi + 1) * 128, :],
                        cout_d[gi][0:128, :])

            # ---- head ----
            nc.sync.dma_start(pin_d[:, :], maxp[:])
            if no_cc:
                for _c in range(NC):
                    nc.sync.dma_start(pout_d[_c * 128:(_c + 1) * 128, :],
                                      pin_d[:, :])
            else:
                nc.gpsimd.collective_compute(
                    "AllGather", ALU.bypass, replica_groups=all_groups,
                    ins=[pin_d[:, :]], outs=[pout_d[:, :]])
            pTf = wpool.tile([128, KC * B], f32, tag="pTf")
            pTf_v = pTf[:].rearrange("p (j b) -> p j b", j=KC)
            for g in range(BGRP):
                c_odd = 2 * g + 1
                nc.sync.dma_start(
                    pTf_v[:, :, g * BL:(g + 1) * BL],
                    pout_d[c_odd * 128:(c_odd + 1) * 128, :].rearrange(
                        "p (j b) -> p j b", j=KC))
            pT = wpool.tile([128, KC * B], bf16, tag="pT")
            nc.scalar.copy(pT[:], pTf[:])

            dps = pst.tile([128, KC * B], f32, tag="dps")
            for mc in range(KC):
                for kc in range(KC):
                    nc.tensor.matmul(
                        dps[:, mc * B:(mc + 1) * B],
                        wd[:, (kc * KC + mc) * 128:(kc * KC + mc + 1) * 128],
                        pT[:, kc * B:(kc + 1) * B],
                        start=(kc == 0), stop=(kc == KC - 1))
            dT = wpool.tile([128, KC * B], bf16, tag="dT")
            nc.scalar.activation(dT[:], dps[:], AF.Relu)

            expl = wpool.tile([B, VS], f32, tag="expl")
            acc = wpool.tile([B, NCH], f32, tag="acc")
            for ch in range(NCH):
                n0 = ch * 512
                nw = min(512, VS - n0)
                wo_c = pool.tile([128, KC * 512], bf16, tag="wo_c")
                wo_cv = wo_c[:].rearrange("p (j n) -> p j n", j=KC)
                for kc in range(KC):
                    nc.sync.dma_start(wo_cv[:, kc, 0:nw],
                                      wo_d[:, kc * VS + n0:kc * VS + n0 + nw])
                lp = pst.tile([B, 512], f32, tag="lp")
                for kc in range(KC):
                    nc.tensor.matmul(
                        lp[:, 0:nw], dT[:, kc * B:(kc + 1) * B],
                        wo_cv[:, kc, 0:nw],
                        start=(kc == 0), stop=(kc == KC - 1))
                nc.scalar.activation(expl[:, n0:n0 + nw], lp[:, 0:nw], AF.Exp,
                                     accum_out=acc[:, ch:ch + 1])
            sums = pool.tile([B, 1], f32, tag="sums")
            nc.vector.tensor_reduce(sums[:], acc[:], axis=mybir.AxisListType.X,
                                    op=ALU.add)
            nc.sync.dma_start(sin_d[:, :], sums[:])
            if no_cc:
                nc.sync.dma_start(sout_d[:, :], sin_d[:, :])
            else:
                nc.gpsimd.collective_compute(
                    "AllReduce", ALU.add, replica_groups=all_groups,
                    ins=[sin_d[:, :]], outs=[sout_d[:, :]])
            tsum = pool.tile([B, 1], f32, tag="tsum")
            nc.sync.dma_start(tsum[:], sout_d[:, :])
            inv = pool.tile([B, 1], f32, tag="inv")
            nc.vector.reciprocal(inv[:], tsum[:])
            prob = wpool.tile([B, VS], f32, tag="prob")
            nc.vector.tensor_scalar_mul(prob[:], expl[:], inv[:])
            nc.sync.dma_start(pr_d[:, :], prob[:])
    nc.finalize()
    return nc


# --------------------------------------------------------------------------
# host prep
# --------------------------------------------------------------------------

def _perm_gates(w):
    i, f, g, o = np.split(w, 4, axis=-1)
    return np.concatenate([i, f, o, g], axis=-1)


def _tile_lhsT(w):
    K, G = w.shape
    kc, mc = K // 128, G // 128
    return np.ascontiguousarray(
        w.reshape(kc, 128, mc, 128).transpose(1, 0, 2, 3).reshape(128, kc * mc * 128)
    ).astype(bf)


def _prep_ids(x_grp):
    # token order (block j, step s, batch b): ids[lane, tile]
    m = x_grp.reshape(BL, NBLK, SB).transpose(1, 2, 0).reshape(-1)
    return np.ascontiguousarray(m.reshape(NGATH, 128).T).astype(np.int32)


def _goff(is_odd):
    lanes = np.arange(128, dtype=np.int32).reshape(128, 1)
    cols = []
    for s in range(NSUP):
        if not is_odd:
            base = s * 128 if s < NBLK else ZROW
        else:
            base = ZROW if s < 2 else CCROW + (s % 2) * 128
        cols.append(base + lanes)
    return np.concatenate(cols, axis=1)


# --------------------------------------------------------------------------
# cached PJRT runner (same mechanics as v1)
# --------------------------------------------------------------------------

def _make_runner(nc):
    import jax
    from jax.experimental.shard_map import shard_map
    from jax.sharding import Mesh, PartitionSpec
    from concourse import bass2jax

    bass2jax.install_neuronx_cc_hook()

    in_names, out_names, out_avals = [], [], []
    partition_name = nc.partition_id_tensor.name if nc.partition_id_tensor else None
    for alloc in nc.m.functions[0].allocations:
        if not isinstance(alloc, mybir.MemoryLocationSet):
            continue
        name = alloc.memorylocations[0].name
        if alloc.kind == "ExternalInput":
            if name != partition_name:
                in_names.append(name)
        elif alloc.kind == "ExternalOutput":
            out_names.append(name)
            out_avals.append(jax.core.ShapedArray(tuple(alloc.tensor_shape),
                                                  mybir.dt.np(alloc.dtype)))
    n_params = len(in_names)
    n_outs = len(out_avals)
    all_in_names = list(in_names) + list(out_names)
    if partition_name is not None:
        all_in_names.append(partition_name)
    donate = tuple(range(n_params, n_params + n_outs))

    def _body(*args):
        operands = list(args)
        if partition_name is not None:
            operands.append(bass2jax.partition_id_tensor())
        outs = bass2jax._bass_exec_p.bind(
            *operands,
            out_avals=tuple(out_avals),
            in_names=tuple(all_in_names),
            out_names=tuple(out_names),
            lowering_input_output_aliases=(),
            sim_require_finite=True,
            sim_require_nnan=True,
            nc=nc,
        )
        return tuple(outs)

    devices = jax.devices()[:NC]
    mesh = Mesh(np.asarray(devices), ("core",))
    in_specs = (PartitionSpec("core"),) * (n_params + n_outs)
    out_specs = (PartitionSpec("core"),) * n_outs
    sharded = jax.jit(
        shard_map(_body, mesh=mesh, in_specs=in_specs, out_specs=out_specs,
                  check_rep=False),
        donate_argnums=donate, keep_unused=True)

    def run(in_maps):
        concat_in = [np.concatenate([np.asarray(m[n]) for m in in_maps], axis=0)
                     for n in in_names]
        zeros = [np.zeros((NC * a.shape[0], *a.shape[1:]), a.dtype)
                 for a in out_avals]
        out_arrs = sharded(*concat_in, *zeros)
        return [
            {n: np.asarray(out_arrs[i]).reshape(NC, *out_avals[i].shape)[c]
             for i, n in enumerate(out_names)}
            for c in range(NC)
        ]

    return run


_CACHE = {}


def _prep_in_maps(x, emb, W1, U1, W2, U2, Wd, Wo):
    w1p = np.concatenate([W1, np.zeros((M - D, 4 * M), np.float32)], axis=0)
    w1t = _tile_lhsT(_perm_gates(w1p))
    u1t = _tile_lhsT(_perm_gates(U1))
    w2t = _tile_lhsT(_perm_gates(W2))
    u2t = _tile_lhsT(_perm_gates(U2))
    wdt = _tile_lhsT(Wd)
    zero_ids = np.zeros((128, NGATH), np.int32)
    goff_e, goff_o = _goff(False), _goff(True)

    ins = []
    for c in range(NC):
        g = c // 2
        odd = c % 2 == 1
        wos = Wo[:, c * VS:(c + 1) * VS]
        wot = np.ascontiguousarray(
            wos.reshape(KC, 128, VS).transpose(1, 0, 2).reshape(128, KC * VS)
        ).astype(bf)
        ins.append({
            "ids": zero_ids if odd else _prep_ids(x[g * BL:(g + 1) * BL]),
            "emb": emb,
            "goff": goff_o if odd else goff_e,
            "ut": u2t if odd else u1t,
            "wt": w2t if odd else w1t,
            "wdt": wdt,
            "wot": wot,
        })
    return ins


def kernel(x, emb, W1, U1, b1, W2, U2, b2, Wd, bd, Wo, bo):
    x = np.asarray(x)
    assert x.dtype == np.int32
    for b_ in (b1, b2, bd, bo):
        assert not np.asarray(b_).any(), "nonzero biases not supported"

    ins = _prep_in_maps(
        x, np.asarray(emb, np.float32),
        np.asarray(W1, np.float32), np.asarray(U1, np.float32),
        np.asarray(W2, np.float32), np.asarray(U2, np.float32),
        np.asarray(Wd, np.float32), np.asarray(Wo, np.float32))

    if "fused" not in _CACHE:
        _CACHE["fused"] = _make_runner(build_fused())
    res = _CACHE["fused"](ins)
    probs = np.concatenate([res[c]["probs"] for c in range(NC)], axis=1)
    return probs.astype(np.float32)


def measure_hw_ns(inputs, measure):
    """HW exec time (ns) of the fused kernel, via R-replicated-NEFF diff."""
    ins = _prep_in_maps(
        np.asarray(inputs["x"]), np.asarray(inputs["emb"], np.float32),
        np.asarray(inputs["W1"], np.float32), np.asarray(inputs["U1"], np.float32),
        np.asarray(inputs["W2"], np.float32), np.asarray(inputs["U2"], np.float32),
        np.asarray(inputs["Wd"], np.float32), np.asarray(inputs["Wo"], np.float32))
    return measure(build_fused, NC, ins, 1, 5, label="fused")


# revision 7
# speedup vs baseline: 2.7940x; 2.7940x over previous
"""Trainium2 Bass kernel v2 for nn_LSTMModel: single fused NEFF.

Layer-pipelined across core pairs: even core 2g runs LSTM layer 1 for batch
group g (16 rows), odd core 2g+1 runs layer 2 for the same rows, lagging two
32-step blocks. h1 blocks move between pair cores via a per-block 2-core
AllGather (DRAM), with a DRAM->DRAM bounce into a unified "reg" region from
which every core fetches its per-block GEMM input by an indirect DMA whose
row offsets are per-core *data* (even cores point at their own staged eT
blocks, odd cores at the collective landing slot) — keeping the SPMD
program identical across cores.

Per step the recurrent matmul is weight-stationary over U tiles in kc-major
order (all 16 gate tiles for hidden chunk 0, then chunk 1, ...), and the
gate elementwise is split into two hidden-chunk halves so the next step's
first kc passes overlap the second half's computation.

The head (global max pool -> dense+relu -> softmax over 50000) is fused in:
AllGather of the pooled h2 over all 8 cores, redundant dense, vocab-sharded
exp+partial sums, AllReduce of the sums, local scaling. Output: the core's
[64, 6250] probability shard.

All matmuls bf16 with fp32 PSUM. Biases are all zero (asserted).
"""

import numpy as np
import ml_dtypes

import concourse.bass as bass
import concourse.bacc as bacc
import concourse.mybir as mybir
import concourse.tile as tile
from concourse.masks import make_identity

bf16 = mybir.dt.bfloat16
f32 = mybir.dt.float32
i32 = mybir.dt.int32
AF = mybir.ActivationFunctionType
ALU = mybir.AluOpType
bf = ml_dtypes.bfloat16

B, T, V, D, M = 64, 512, 50000, 128, 512
NC = 8
BGRP = 4                 # batch groups (core pairs)
BL = B // BGRP           # 16 batch rows per pair
VS = V // NC             # 6250 vocab cols per core
SB = 32                  # steps per block
KC = M // 128            # 4 hidden chunks
MC = 4 * M // 128        # 16 gate chunks
NBLK = T // SB           # 16 blocks
NSUP = NBLK + 2          # 18 supersteps (odd cores lag by 2)
NTOK = BL * T            # 8192 tokens per group
NGATH = NTOK // 128      # 64 gather tiles
GPB = NGATH // NBLK      # 4 gather tiles per block
BW = KC * SB * BL        # 2048: cols of one block buffer [128, kc, s, b]
ZROW = NBLK * 128        # reg row base of the zero block
CCROW = (NBLK + 1) * 128  # reg row base of cc landing slots (2 x 128 rows)
NREG = CCROW + 2 * 128


def _new_nc():
    return bacc.Bacc("TRN2", target_bir_lowering=False, debug=False, num_devices=NC)


def build_fused(reps=1, no_cc=False):
    NCH = (VS + 511) // 512
    nc = _new_nc()
    ids_d = nc.dram_tensor("ids", [128, NGATH], i32, kind="ExternalInput")
    emb_d = nc.dram_tensor("emb", [V, D], f32, kind="ExternalInput")
    goff_d = nc.dram_tensor("goff", [128, NSUP], i32, kind="ExternalInput")
    u_d = nc.dram_tensor("ut", [128, KC * MC * 128], bf16, kind="ExternalInput")
    w_d = nc.dram_tensor("wt", [128, KC * MC * 128], bf16, kind="ExternalInput")
    wd_d = nc.dram_tensor("wdt", [128, KC * KC * 128], bf16, kind="ExternalInput")
    wo_d = nc.dram_tensor("wot", [128, KC * VS], bf16, kind="ExternalInput")
    pr_d = nc.dram_tensor("probs", [B, VS], f32, kind="ExternalOutput")

    reg = nc.dram_tensor("reg", [NREG, BW], bf16, kind="Internal")
    cin_d = nc.dram_tensor("cc_in", [2 * 128, BW], bf16, kind="Internal")
    cout_d = [nc.dram_tensor(f"cc_out{i}", [2 * 128, BW], bf16, kind="Internal")
              for i in range(2)]
    pin_d = nc.dram_tensor("p_in", [128, KC * BL], f32, kind="Internal")
    pout_d = nc.dram_tensor("p_all", [NC * 128, KC * BL], f32, kind="Internal")
    sin_d = nc.dram_tensor("s_in", [B, 1], f32, kind="Internal")
    sout_d = nc.dram_tensor("s_all", [B, 1], f32, kind="Internal")

    pair_groups = [[2 * g, 2 * g + 1] for g in range(BGRP)]
    all_groups = [list(range(NC))]

    with tile.TileContext(nc) as tc:
        with tc.tile_pool(name="wts", bufs=1) as wpool, \
             tc.tile_pool(name="sb", bufs=3) as pool, \
             tc.tile_pool(name="st", bufs=2) as spool, \
             tc.tile_pool(name="ps", bufs=2, space="PSUM") as psp, \
             tc.tile_pool(name="pst", bufs=1, space="PSUM") as pst:
          for _rep in range(reps):
            u = wpool.tile([128, KC * MC * 128], bf16, tag="u")
            w = wpool.tile([128, KC * MC * 128], bf16, tag="w")
            wd = wpool.tile([128, KC * KC * 128], bf16, tag="wd")
            hist = [wpool.tile([128, BW], bf16, tag=f"hist{i}", name=f"hist{i}")
                    for i in range(2)]
            xw = [wpool.tile([128, MC * SB * BL], bf16, tag=f"xw{i}",
                             name=f"xw{i}") for i in range(2)]
            gemin = [wpool.tile([128, BW], bf16, tag=f"gemin{i}",
                                name=f"gemin{i}") for i in range(2)]
            cst = wpool.tile([128, KC * BL], f32, tag="cst")
            maxp = wpool.tile([128, KC * BL], f32, tag="maxp")
            zero = wpool.tile([128, BW], bf16, tag="zero")
            ident = wpool.tile([128, 128], f32, tag="ident")
            ids_t = wpool.tile([128, NGATH], i32, tag="ids")
            goff_t = wpool.tile([128, NSUP], i32, tag="goff")

            nc.sync.dma_start(u[:], u_d[:])
            nc.sync.dma_start(w[:], w_d[:])
            nc.sync.dma_start(wd[:], wd_d[:])
            nc.sync.dma_start(ids_t[:], ids_d[:])
            nc.sync.dma_start(goff_t[:], goff_d[:])
            nc.vector.memset(cst[:], 0.0)
            nc.vector.memset(maxp[:], 0.0)
            nc.vector.memset(zero[:], 0.0)
            nc.vector.memset(hist[1][:], 0.0)   # h_{-1} = 0 slot
            make_identity(nc, ident[:])
            ident_b = wpool.tile([128, 128], bf16, tag='identb')
            nc.scalar.copy(ident_b[:], ident[:])

            # reg zero block + first eT block + its pad area
            nc.sync.dma_start(reg[ZROW:ZROW + 128, :], zero[:])

            def stage_block(j):
                """Gather+transpose eT block j into reg rows [j*128, +128)."""
                for t in range(GPB):
                    et = pool.tile([128, 128], f32, tag="gath")
                    g = j * GPB + t
                    nc.gpsimd.indirect_dma_start(
                        out=et[:], out_offset=None, in_=emb_d[:],
                        in_offset=bass.IndirectOffsetOnAxis(
                            ap=ids_t[:, g:g + 1], axis=0))
                    tp = pst.tile([128, 128], f32, tag="tp")
                    nc.tensor.transpose(out=tp[:], in_=et[:], identity=ident[:])
                    sg = pool.tile([128, 128], bf16, tag="sg")
                    nc.scalar.copy(sg[:], tp[:])
                    nc.sync.dma_start(
                        reg[j * 128:(j + 1) * 128, t * 128:(t + 1) * 128], sg[:])
                # zero-fill kc chunks 1..3 of the block (never real data there)
                nc.sync.dma_start(
                    reg[j * 128:(j + 1) * 128, SB * BL:BW],
                    zero[:, 0:BW - SB * BL])

            stage_block(0)
            stage_block(1)

            hist_v = [h[:].rearrange("p (j s b) -> p j s b", j=KC, s=SB)
                      for h in hist]
            xw_v = [x[:].rearrange("p (j g s b) -> p j g s b", j=KC, g=4, s=SB)
                    for x in xw]
            gem_v = [x[:].rearrange("p (j s b) -> p j s b", j=KC, s=SB)
                     for x in gemin]
            cst_v = cst[:].rearrange("p (j b) -> p j b", j=KC)
            maxp_v = maxp[:].rearrange("p (j b) -> p j b", j=KC)

            def gather(s):
                nc.gpsimd.indirect_dma_start(
                    out=gemin[s % 2][:], out_offset=None, in_=reg[:, :],
                    in_offset=bass.IndirectOffsetOnAxis(
                        ap=goff_t[:, s:s + 1], axis=0))

            gather(0)

            def gemm_units(s):
                """Emit-list of closures: 4 kc-MMs + 1 copy per mc chunk."""
                gi = s % 2
                units = []
                state = {}
                for mc in range(MC):
                    def mk(mc):
                        def mm(kc):
                            def f():
                                if kc == 0:
                                    state[mc] = psp.tile(
                                        [128, SB * BL], f32, tag="gemm",
                                        name=f"gp_{mc}")
                                nc.tensor.matmul(
                                    state[mc][:],
                                    w[:, (kc * MC + mc) * 128:
                                      (kc * MC + mc + 1) * 128],
                                    gem_v[gi][:, kc, :, :],
                                    start=(kc == 0), stop=(kc == KC - 1))
                            return f
                        def cp():
                            nc.scalar.copy(
                                xw_v[gi][:, mc % KC, mc // KC, :, :],
                                state[mc][:].rearrange("p (s b) -> p s b", s=SB))
                        return [mm(kc) for kc in range(KC)] + [cp]
                    units.extend(mk(mc))
                return units

            # pass-4 mc order: A-half hidden chunks (j in 0,1) first
            P4 = [g * KC + j for j in (0, 1) for g in range(4)] + \
                 [g * KC + j for j in (2, 3) for g in range(4)]

            def lstm_step(s, st, fill):
                gi = s % 2
                if st == 0:
                    hp = hist_v[1 - gi]
                    hrow = SB - 1
                else:
                    hp = hist_v[gi]
                    hrow = st - 1
                zp = psp.tile([128, MC * BL], f32, tag="zp")
                zp_v = zp[:].rearrange("p (j g b) -> p j g b", j=KC, g=4)
                for kc in range(KC):
                    order = range(MC) if kc < KC - 1 else P4
                    for mc in order:
                        g, j = mc // KC, mc % KC
                        nc.tensor.matmul(
                            zp_v[:, j, g, :],
                            u[:, (kc * MC + mc) * 128:(kc * MC + mc + 1) * 128],
                            hp[:, kc, hrow, :],
                            start=(kc == 0), stop=False)
                # fold xw into PSUM via identity matmuls (A half then B half)
                for (j0, j1) in ((0, 2), (2, 4)):
                    nc.tensor.matmul(
                        zp[:, j0 * 4 * BL:j1 * 4 * BL], ident_b[:],
                        xw_v[gi][:, j0:j1, :, st, :], start=False, stop=True)
                for (j0, j1) in ((0, 2), (2, 4)):
                    jn = j1 - j0
                    sig = spool.tile([128, 3 * jn * BL], f32, tag=f"sig{j0}")
                    sig_v = sig[:].rearrange("p (j g b) -> p j g b", j=jn, g=3)
                    nc.scalar.activation(sig_v[:, :, :, :],
                                         zp_v[:, j0:j1, 0:3, :], AF.Sigmoid)
                    ig = spool.tile([128, jn * BL], f32, tag=f"ig{j0}")
                    ig_v = ig[:].rearrange("p (j b) -> p j b", j=jn)
                    nc.vector.scalar_tensor_tensor(
                        out=ig_v[:, :, :], in0=zp_v[:, j0:j1, 3, :], scalar=0.0,
                        in1=sig_v[:, :, 0, :], op0=ALU.max, op1=ALU.mult)
                    fc = spool.tile([128, jn * BL], f32, tag=f"fc{j0}")
                    fc_v = fc[:].rearrange("p (j b) -> p j b", j=jn)
                    nc.gpsimd.tensor_tensor(
                        out=fc_v[:, :, :], in0=sig_v[:, :, 1, :],
                        in1=cst_v[:, j0:j1, :], op=ALU.mult)
                    nc.vector.tensor_tensor(
                        out=cst_v[:, j0:j1, :], in0=fc_v[:, :, :],
                        in1=ig_v[:, :, :], op=ALU.add)
                    nc.vector.scalar_tensor_tensor(
                        out=hist_v[gi][:, j0:j1, st, :],
                        in0=cst_v[:, j0:j1, :], scalar=0.0,
                        in1=sig_v[:, :, 2, :], op0=ALU.max, op1=ALU.mult)

            FILL_ST = 8  # first step that takes next-superstep GEMM fill work

            units0 = gemm_units(0)
            for f in units0:
                f()

            for s in range(NSUP):
                gi = s % 2
                if s + 1 < NSUP:
                    gather(s + 1)
                if s > 0:
                    for f in gemm_units(s):
                        f()
                for st in range(SB):
                    lstm_step(s, st, [])
                # fold this block's h into the running max (pool over s)
                bmax = spool.tile([128, KC * BL], f32, tag="bmax")
                nc.vector.tensor_reduce(
                    bmax[:].rearrange("p (j b) -> p j b", j=KC),
                    hist[gi][:].rearrange("p (j s b) -> p j b s", j=KC, s=SB),
                    axis=mybir.AxisListType.X, op=ALU.max)
                nc.vector.tensor_tensor(out=maxp[:], in0=maxp[:],
                                        in1=bmax[:], op=ALU.max)
                if s < NBLK:
                    nc.sync.dma_start(cin_d[gi * 128:(gi + 1) * 128, :],
                                      hist[gi][:])
                    if no_cc:
                        nc.sync.dma_start(cout_d[gi][0:128, :],
                                          cin_d[gi * 128:(gi + 1) * 128, :])
                        nc.sync.dma_start(cout_d[gi][128:256, :],
                                          cin_d[gi * 128:(gi + 1) * 128, :])
                    else:
                        nc.gpsimd.collective_compute(
                            "AllGather", ALU.bypass, replica_groups=pair_groups,
                            ins=[cin_d[gi * 128:(gi + 1) * 128, :]],
                            outs=[cout_d[gi][:, :]])
                if s + 2 < NBLK:
                    stage_block(s + 2)
                if s < NBLK:
                    # bounce rank-0 slice of CC_s into the reg landing slot;
                    # consumed by the partner's gather at superstep s+2
                    nc.sync.dma_start(
                        reg[CCROW + gi * 128:CCROW + (gi + 1) * 128, :],
                        cout_d[gi][0:128, :])

            # ---- head ----
            nc.sync.dma_start(pin_d[:, :], maxp[:])
            if no_cc:
                for _c in range(NC):
                    nc.sync.dma_start(pout_d[_c * 128:(_c + 1) * 128, :],
                                      pin_d[:, :])
            else:
                nc.gpsimd.collective_compute(
                    "AllGather", ALU.bypass, replica_groups=all_groups,
                    ins=[pin_d[:, :]], outs=[pout_d[:, :]])
            pTf = wpool.tile([128, KC * B], f32, tag="pTf")
            pTf_v = pTf[:].rearrange("p (j b) -> p j b", j=KC)
            for g in range(BGRP):
                c_odd = 2 * g + 1
                nc.sync.dma_start(
                    pTf_v[:, :, g * BL:(g + 1) * BL],
                    pout_d[c_odd * 128:(c_odd + 1) * 128, :].rearrange(
                        "p (j b) -> p j b", j=KC))
            pT = wpool.tile([128, KC * B], bf16, tag="pT")
            nc.scalar.copy(pT[:], pTf[:])

            dps = pst.tile([128, KC * B], f32, tag="dps")
            for mc in range(KC):
                for kc in range(KC):
                    nc.tensor.matmul(
                        dps[:, mc * B:(mc + 1) * B],
                        wd[:, (kc * KC + mc) * 128:(kc * KC + mc + 1) * 128],
                        pT[:, kc * B:(kc + 1) * B],
                        start=(kc == 0), stop=(kc == KC - 1))
            dT = wpool.tile([128, KC * B], bf16, tag="dT")
            nc.scalar.activation(dT[:], dps[:], AF.Relu)

            expl = wpool.tile([B, VS], f32, tag="expl")
            acc = wpool.tile([B, NCH], f32, tag="acc")
            for ch in range(NCH):
                n0 = ch * 512
                nw = min(512, VS - n0)
                wo_c = pool.tile([128, KC * 512], bf16, tag="wo_c")
                wo_cv = wo_c[:].rearrange("p (j n) -> p j n", j=KC)
                for kc in range(KC):
                    nc.sync.dma_start(wo_cv[:, kc, 0:nw],
                                      wo_d[:, kc * VS + n0:kc * VS + n0 + nw])
                lp = pst.tile([B, 512], f32, tag="lp")
                for kc in range(KC):
                    nc.tensor.matmul(
                        lp[:, 0:nw], dT[:, kc * B:(kc + 1) * B],
                        wo_cv[:, kc, 0:nw],
                        start=(kc == 0), stop=(kc == KC - 1))
                nc.scalar.activation(expl[:, n0:n0 + nw], lp[:, 0:nw], AF.Exp,
                                     accum_out=acc[:, ch:ch + 1])
            sums = pool.tile([B, 1], f32, tag="sums")
            nc.vector.tensor_reduce(sums[:], acc[:], axis=mybir.AxisListType.X,
                                    op=ALU.add)
            nc.sync.dma_start(sin_d[:, :], sums[:])
            if no_cc:
                nc.sync.dma_start(sout_d[:, :], sin_d[:, :])
            else:
                nc.gpsimd.collective_compute(
                    "AllReduce", ALU.add, replica_groups=all_groups,
                    ins=[sin_d[:, :]], outs=[sout_d[:, :]])
            tsum = pool.tile([B, 1], f32, tag="tsum")
            nc.sync.dma_start(tsum[:], sout_d[:, :])
            inv = pool.tile([B, 1], f32, tag="inv")
            nc.vector.reciprocal(inv[:], tsum[:])
            prob = wpool.tile([B, VS], f32, tag="prob")
            nc.vector.tensor_scalar_mul(prob[:], expl[:], inv[:])
            nc.sync.dma_start(pr_d[:, :], prob[:])
    nc.finalize()
    return nc


# --------------------------------------------------------------------------
# host prep
# --------------------------------------------------------------------------

def _perm_gates(w):
    i, f, g, o = np.split(w, 4, axis=-1)
    return np.concatenate([i, f, o, g], axis=-1)


def _tile_lhsT(w):
    K, G = w.shape
    kc, mc = K // 128, G // 128
    return np.ascontiguousarray(
        w.reshape(kc, 128, mc, 128).transpose(1, 0, 2, 3).reshape(128, kc * mc * 128)
    ).astype(bf)


def _prep_ids(x_grp):
    # token order (block j, step s, batch b): ids[lane, tile]
    m = x_grp.reshape(BL, NBLK, SB).transpose(1, 2, 0).reshape(-1)
    return np.ascontiguousarray(m.reshape(NGATH, 128).T).astype(np.int32)


def _goff(is_odd):
    lanes = np.arange(128, dtype=np.int32).reshape(128, 1)
    cols = []
    for s in range(NSUP):
        if not is_odd:
            base = s * 128 if s < NBLK else ZROW
        else:
            base = ZROW if s < 2 else CCROW + (s % 2) * 128
        cols.append(base + lanes)
    return np.concatenate(cols, axis=1)


# --------------------------------------------------------------------------
# cached PJRT runner (same mechanics as v1)
# --------------------------------------------------------------------------

def _make_runner(nc):
    import jax
    from jax.experimental.shard_map import shard_map
    from jax.sharding import Mesh, PartitionSpec
    from concourse import bass2jax

    bass2jax.install_neuronx_cc_hook()

    in_names, out_names, out_avals = [], [], []
    partition_name = nc.partition_id_tensor.name if nc.partition_id_tensor else None
    for alloc in nc.m.functions[0].allocations:
        if not isinstance(alloc, mybir.MemoryLocationSet):
            continue
        name = alloc.memorylocations[0].name
        if alloc.kind == "ExternalInput":
            if name != partition_name:
                in_names.append(name)
        elif alloc.kind == "ExternalOutput":
            out_names.append(name)
            out_avals.append(jax.core.ShapedArray(tuple(alloc.tensor_shape),
                                                  mybir.dt.np(alloc.dtype)))
    n_params = len(in_names)
    n_outs = len(out_avals)
    all_in_names = list(in_names) + list(out_names)
    if partition_name is not None:
        all_in_names.append(partition_name)
    donate = tuple(range(n_params, n_params + n_outs))

    def _body(*args):
        operands = list(args)
        if partition_name is not None:
            operands.append(bass2jax.partition_id_tensor())
        outs = bass2jax._bass_exec_p.bind(
            *operands,
            out_avals=tuple(out_avals),
            in_names=tuple(all_in_names),
            out_names=tuple(out_names),
            lowering_input_output_aliases=(),
            sim_require_finite=True,
            sim_require_nnan=True,
            nc=nc,
        )
        return tuple(outs)

    devices = jax.devices()[:NC]
    mesh = Mesh(np.asarray(devices), ("core",))
    in_specs = (PartitionSpec("core"),) * (n_params + n_outs)
    out_specs = (PartitionSpec("core"),) * n_outs
    sharded = jax.jit(
        shard_map(_body, mesh=mesh, in_specs=in_specs, out_specs=out_specs,
                  check_rep=False),
        donate_argnums=donate, keep_unused=True)

    def run(in_maps):
        concat_in = [np.concatenate([np.asarray(m[n]) for m in in_maps], axis=0)
                     for n in in_names]
        zeros = [np.zeros((NC * a.shape[0], *a.shape[1:]), a.dtype)
                 for a in out_avals]
        out_arrs = sharded(*concat_in, *zeros)
        return [
            {n: np.asarray(out_arrs[i]).reshape(NC, *out_avals[i].shape)[c]
             for i, n in enumerate(out_names)}
            for c in range(NC)
        ]

    return run


_CACHE = {}


def _prep_in_maps(x, emb, W1, U1, W2, U2, Wd, Wo):
    w1p = np.concatenate([W1, np.zeros((M - D, 4 * M), np.float32)], axis=0)
    w1t = _tile_lhsT(_perm_gates(w1p))
    u1t = _tile_lhsT(_perm_gates(U1))
    w2t = _tile_lhsT(_perm_gates(W2))
    u2t = _tile_lhsT(_perm_gates(U2))
    wdt = _tile_lhsT(Wd)
    zero_ids = np.zeros((128, NGATH), np.int32)
    goff_e, goff_o = _goff(False), _goff(True)

    ins = []
    for c in range(NC):
        g = c // 2
        odd = c % 2 == 1
        wos = Wo[:, c * VS:(c + 1) * VS]
        wot = np.ascontiguousarray(
            wos.reshape(KC, 128, VS).transpose(1, 0, 2).reshape(128, KC * VS)
        ).astype(bf)
        ins.append({
            "ids": zero_ids if odd else _prep_ids(x[g * BL:(g + 1) * BL]),
            "emb": emb,
            "goff": goff_o if odd else goff_e,
            "ut": u2t if odd else u1t,
            "wt": w2t if odd else w1t,
            "wdt": wdt,
            "wot": wot,
        })
    return ins


def kernel(x, emb, W1, U1, b1, W2, U2, b2, Wd, bd, Wo, bo):
    x = np.asarray(x)
    assert x.dtype == np.int32
    for b_ in (b1, b2, bd, bo):
        assert not np.asarray(b_).any(), "nonzero biases not supported"

    ins = _prep_in_maps(
        x, np.asarray(emb, np.float32),
        np.asarray(W1, np.float32), np.asarray(U1, np.float32),
        np.asarray(W2, np.float32), np.asarray(U2, np.float32),
        np.asarray(Wd, np.float32), np.asarray(Wo, np.float32))

    if "fused" not in _CACHE:
        _CACHE["fused"] = _make_runner(build_fused())
    res = _CACHE["fused"](ins)
    probs = np.concatenate([res[c]["probs"] for c in range(NC)], axis=1)
    return probs.astype(np.float32)


def measure_hw_ns(inputs, measure):
    """HW exec time (ns) of the fused kernel, via R-replicated-NEFF diff."""
    ins = _prep_in_maps(
        np.asarray(inputs["x"]), np.asarray(inputs["emb"], np.float32),
        np.asarray(inputs["W1"], np.float32), np.asarray(inputs["U1"], np.float32),
        np.asarray(inputs["W2"], np.float32), np.asarray(inputs["U2"], np.float32),
        np.asarray(inputs["Wd"], np.float32), np.asarray(inputs["Wo"], np.float32))
    return measure(build_fused, NC, ins, 1, 5, label="fused")
